# revision 13
# baseline (speedup 1.0000x reference)
"""BasicRGCN Trainium2 kernel (8 NeuronCores, SPMD) - AllReduce formulation.

Math (reference):
    x = features                                   # [N, F]
    for l in 0..1:
        y = sum_r A[r] @ x @ W[l, r].T             # [N, F]
        x = sigmoid(y)
    out[r] = (x @ M_r) @ x.T                       # [R, N, N]

Layer 2 is computed as a PARTIAL SUM per core: core c holds both its
adjacency row-slab A[:, rows_c, :] (for layer 1 + its output slab) and an
adjacency column-slab A[:, :, cols_c]. After layer 1 each core projects only
its LOCAL x1 rows (h2loc = x1_loc @ W2.T, fp8) and computes
y2_partial[g, n] = sum_r sum_{m in cols_c} h2loc_r[m, g] A[r, n, m] for ALL
n. A fp16 ReduceScatter of y2_partial gives each core the summed g-rows of
its shard; the sigmoid runs once per shard and a fp16 AllGather of the
activated x2 reconstructs the full [F, N] x2 on every core - replacing the
two AllGathers (x1 and x2) of the row-parallel formulation, removing the
gathered-h2 recompute from the critical path, and paying much less than a
full AllReduce. Partials are pre-scaled by 1/4 (folded into W2 on the
host) so the fp16 CCE sum cannot overflow; the sigmoid applies scale=4.

Per-core identity (which columns are "mine" for the DistMult xm factor)
enters only through a tiny per-core int32 index input driving one indirect
DMA gather - the NEFF itself is identical on all 8 cores (SPMD).

Precision: fp8 adjacency/h1/h2loc with fp32 PSUM accumulation (layer-2
pre-activations are ~5e4 so sigmoid saturates and absorbs layer error);
fp16 x2/xm DistMult (worst-case rel err ~4e-5 in this regime).

Schedule notes (from ntff profiles of prior revisions):
  * The collectives runtime pays a ~40us init/launch-skew barrier (starts
    ~22us into the NEFF regardless of trigger time) plus ~11us first-op
    setup; both overlap the load/compute phase, so the ReduceScatter starts
    processing about when its data is ready.
  * Loads (17 MiB: row-slab, col-slab, h1) rotate across the three DGE
    rings (sync=SP-HWDGE, gpsimd=SWDGE, scalar=ACT-HWDGE); the phase is
    HBM-bound (~358 GB/s) at ~50us, overlapping L1 and the cc barrier.
  * Layer 1 and the layer-2 partials interleave 4 independent PSUM
    accumulation chains (a single chain serializes at ~390ns/matmul; 4
    chains pipeline at ~234ns), consuming slab blocks in DMA arrival
    order.
  * Output stores are one fully-contiguous 2 MiB DMA per 128-row block,
    rotated across the three DGE rings (~350 GB/s sustained, HBM-write
    bound - the phase floor).
  * No keep-warm is needed: the PE idles only across the AllReduce, and
    even at the cold-isolated matmul rate the DistMult (one K=64 fp16
    matmul per [128,512] tile) outpaces the stores.
"""

import numpy as np
import ml_dtypes

import concourse.bacc as bacc
import concourse.mybir as mybir
import concourse.tile as tile
from concourse import bass, bass_utils

R, N, F = 4, 4096, 64
NCORES = 8
NL = N // NCORES          # 512 local node rows per core
MB = N // 128             # 32 contraction blocks of 128 (layer 1)
MBL = NL // 128           # 4 local contraction blocks (layer 2 partial)
NB = NL // 128            # 4 output row-blocks per core
MC = N // 512             # 8 column-chunks

WARM0 = 16                # pre-warm matmuls at kernel start
Y2SCALE = 4.0             # fp16 AR partials carry y2/4; sigmoid re-scales

F8NP = ml_dtypes.float8_e4m3fn
F8 = mybir.dt.float8e4
F16 = mybir.dt.float16
F32 = mybir.dt.float32
I32 = mybir.dt.int32

# Set by the test harness to collect a profile; grading path leaves these alone.
TRACE = False
LAST_RESULT = None

_NC_CACHE = None


def _build():
    nc = bacc.Bacc("TRN2", target_bir_lowering=False, debug=False,
                   num_devices=NCORES)

    # Per-core inputs (host pre-laid-out; see kernel() below).
    atr = nc.dram_tensor("atr", [R, 128, MB, NL], F8, kind="ExternalInput")
    atc = nc.dram_tensor("atc", [R, 128, MBL, N], F8, kind="ExternalInput")
    h1 = nc.dram_tensor("h1", [128, R * MB * F], F8, kind="ExternalInput")
    wt2 = nc.dram_tensor("wt2", [F, R * F], F16, kind="ExternalInput")
    relm = nc.dram_tensor("relm", [F, R * F], F16, kind="ExternalInput")
    idx = nc.dram_tensor("idx", [F, 1], I32, kind="ExternalInput")
    out = nc.dram_tensor("out", [R, NL, N], F32, kind="ExternalOutput")

    rg = [list(range(NCORES))]
    SIG = mybir.ActivationFunctionType.Sigmoid

    with tile.TileContext(nc) as tc:
        with (
            tc.tile_pool(name="big", bufs=1) as big,
            tc.tile_pool(name="sb", bufs=1) as sb,
            tc.tile_pool(name="stage", bufs=3) as stage,
            tc.tile_pool(name="psl", bufs=4, space="PSUM") as psl,
            tc.tile_pool(name="psh", bufs=1, space="PSUM") as psh,
            tc.tile_pool(name="pso", bufs=3, space="PSUM") as pso,
            tc.tile_pool(name="xn", bufs=1) as xn,
            tc.tile_pool(name="dram", bufs=1, space="DRAM") as dram,
        ):
            # Resident adjacency slabs: rows (L1) + columns (L2 partial).
            a_res = big.tile([128, R * MB * NL], F8)
            a_v = a_res.rearrange("p (r mb j) -> p r mb j", r=R, mb=MB)
            a_col = big.tile([128, R * MBL * N], F8)
            ac_v = a_col.rearrange("p (r mb n) -> p r mb n", r=R, mb=MBL)

            h1_sb = sb.tile([128, R * MB * F], F8)
            h1_v = h1_sb.rearrange("p (r mb g) -> p r mb g", r=R, mb=MB)
            wt2_sb = sb.tile([F, R * F], F16)
            relm_sb = sb.tile([F, R * F], F16)
            idx_sb = sb.tile([F, 1], I32)

            rings = [nc.sync, nc.gpsimd, nc.scalar]
            qi = 0

            def ring():
                nonlocal qi
                e = rings[qi % 3]
                qi += 1
                return e

            ring().dma_start(wt2_sb[:], wt2[:])
            ring().dma_start(relm_sb[:], relm[:])
            ring().dma_start(idx_sb[:], idx[:])
            HC = R * MB * F // 4
            for q in range(4):
                ring().dma_start(h1_sb[:, q * HC:(q + 1) * HC],
                                 h1[:, q * HC:(q + 1) * HC])
            # Row-slab first (layer 1 consumes it), then column-slab.
            H = MB // 4
            for r in range(R):
                for h in range(4):
                    ring().dma_start(
                        a_v[:, r, h * H:(h + 1) * H, :],
                        atr[r, :, h * H:(h + 1) * H, :],
                    )
            for r in range(R):
                for mbl in range(MBL):
                    ring().dma_start(
                        ac_v[:, r, mbl, :],
                        atc[r, :, mbl, :],
                    )

            # Pre-warm the PE while the adjacency stream lands. Junk results
            # land in a psl ring buffer; layer 1 resets it with start=True.
            warm = psl.tile([F, NL], F32, tag="acc")
            for _ in range(WARM0):
                nc.tensor.matmul(warm[:], h1_v[:, 0, 0, :],
                                 h1_sb[:, 0:NL], start=True, stop=True)

            # ---- Layer 1: y1T[g, n_local] = sum_{r, m} h1_r[m, g] A[r, n, m]
            # 4 interleaved PSUM accumulation chains (avoids the serialized
            # single-chain matmul pace), combined on the vector engine.
            x1pack = sb.tile([F, NL], F16)
            y1c = [psl.tile([F, NL], F32, tag="acc", name=f"y1c{j}")
                   for j in range(4)]
            NK1 = R * MB
            k = 0
            for r in range(R):
                for mb in range(MB):
                    nc.tensor.matmul(
                        y1c[k % 4][:], h1_v[:, r, mb, :], a_v[:, r, mb, :],
                        start=(k < 4), stop=(k >= NK1 - 4),
                    )
                    k += 1
            y1s = sb.tile([F, NL], F32)
            nc.vector.tensor_copy(y1s[:], y1c[0][:])
            nc.vector.tensor_add(y1s[:], y1s[:], y1c[1][:])
            nc.vector.tensor_add(y1s[:], y1s[:], y1c[2][:])
            nc.vector.tensor_add(y1s[:], y1s[:], y1c[3][:])
            nc.scalar.activation(x1pack[:], y1s[:], SIG)

            # ---- h2loc[m, (r g)] = x1_loc[m, :] @ W2r.T (local rows, fp8)
            h2loc = sb.tile([128, MBL * R * F], F8)
            h2l_v = h2loc.rearrange("p (mb r g) -> p mb r g", mb=MBL, r=R)
            for mbl in range(MBL):
                ph = psh.tile([128, R * F], F32, tag="h")
                nc.tensor.matmul(ph[:], x1pack[:, mbl * 128:(mbl + 1) * 128],
                                 wt2_sb[:], start=True, stop=True)
                ceng = nc.vector.tensor_copy if mbl % 2 == 0 else nc.scalar.copy
                ceng(h2l_v[:, mbl, :, :],
                     ph[:].rearrange("p (r g) -> p r g", r=R))

            # ---- Layer-2 partials: y2p[g, n] = sum_{r, m_loc} h2loc A_col.
            # Two n-halves; 4 interleaved PSUM accumulation chains per half,
            # consuming column-slab blocks in DMA arrival order.
            y2p = xn.tile([F, N], F16, tag="xn", name="y2p")
            for half in range(2):
                acc = [psl.tile([F, 512], F32, tag="acc", name=f"acc{half}_{j}")
                       for j in range(4)]
                k = 0
                for r in range(R):
                    for mbl in range(MBL):
                        for j in range(4):
                            off = half * 2048 + j * 512
                            nc.tensor.matmul(
                                acc[j][:], h2l_v[:, mbl, r, :],
                                ac_v[:, r, mbl, off:off + 512],
                                start=(k == 0), stop=(k == R * MBL - 1),
                            )
                        k += 1
                for j in range(4):
                    off = half * 2048 + j * 512
                    ceng = (nc.vector.tensor_copy if j % 2 == 0
                            else nc.scalar.copy)
                    ceng(y2p[:, off:off + 512], acc[j][:])

            # ---- ReduceScatter y2 partials (fp16, scaled by 1/4): each core
            # receives the summed g-rows [8c:8c+8] for all n (the flat
            # scatter axis of the g-major [F, N] buffer).
            rs_in = dram.tile([F, N], F16)
            rs_out = dram.tile([F // NCORES, N], F16)
            nc.sync.dma_start(rs_in[:], y2p[:])
            nc.gpsimd.collective_compute(
                "ReduceScatter", mybir.AluOpType.add, replica_groups=rg,
                ins=[rs_in[:]], outs=[rs_out[:]],
            )

            # Sigmoid the own shard (scale undoes the 1/4 partial headroom),
            # then AllGather the activated fp16 x2 (rank-major g-rows concat
            # reconstructs x2[g, n] row-major exactly).
            ysh = sb.tile([F, 512], F16)
            nc.sync.dma_start(
                ysh[:], rs_out[:].rearrange("a (b j) -> (a b) j", b=NCORES))
            xsh = sb.tile([F, 512], F16)
            nc.scalar.activation(xsh[:], ysh[:], SIG, scale=Y2SCALE)
            ag_in = dram.tile([F // NCORES, N], F16)
            ag_out = dram.tile([F, N], F16, addr_space="Shared")
            nc.sync.dma_start(
                ag_in[:].rearrange("a (b j) -> (a b) j", b=NCORES), xsh[:])
            nc.gpsimd.collective_compute(
                "AllGather", mybir.AluOpType.bypass, replica_groups=rg,
                ins=[ag_in[:]], outs=[ag_out[:]],
            )

            # ---- Own-columns x2 slice via indirect gather (per-core idx).
            # ag_out viewed as [(g, chunk), 512]; row g*8+c -> partition g.
            x2own = sb.tile([F, NL], F16)
            ag_view = ag_out[:].rearrange("g (c j) -> (g c) j", c=MC)
            nc.gpsimd.indirect_dma_start(
                out=x2own[:], out_offset=None,
                in_=ag_view,
                in_offset=bass.IndirectOffsetOnAxis(ap=idx_sb[:, :1], axis=0),
            )

            # ---- Full x2: straight loads (already activated).
            x2t = xn.tile([F, N], F16, tag="xn", name="x2t")
            for q in range(MC):
                cs = slice(q * 512, (q + 1) * 512)
                rings[q % 3].dma_start(x2t[:, cs], ag_out[:, cs])

            # ---- xmT[r] = (M_r.T @ x2_own) in fp16
            xm16 = sb.tile([F, R * NL], F16)
            xm16_v = xm16.rearrange("g (r j) -> g r j", r=R)
            for r in range(R):
                pxm = psl.tile([F, NL], F32, tag="acc")
                nc.tensor.matmul(pxm[:], relm_sb[:, r * F:(r + 1) * F],
                                 x2own[:], start=True, stop=True)
                ceng = nc.vector.tensor_copy if r % 2 == 0 else nc.scalar.copy
                ceng(xm16_v[:, r, :], pxm[:])

            # ---- DistMult scores: out[r, n, m] = sum_g xm[r][n, g] x2[m, g]
            # One K=64 fp16 matmul per [128, 512] tile; stage a full 128-row
            # block in SBUF, store it as ONE contiguous 2 MiB DMA, rotating
            # across the three DGE rings.
            st_engs = [nc.sync, nc.gpsimd, nc.scalar]
            blk = 0
            for r in range(R):
                for nb in range(NB):
                    lhs = xm16_v[:, r, nb * 128:(nb + 1) * 128]
                    so = stage.tile([128, N], F32, tag="so")
                    for mc in range(MC):
                        cs = slice(mc * 512, (mc + 1) * 512)
                        po = pso.tile([128, 512], F32, tag="o")
                        nc.tensor.matmul(po[:], lhs, x2t[:, cs],
                                         start=True, stop=True)
                        if mc % 2 == 0:
                            nc.vector.tensor_copy(so[:, cs], po[:])
                        else:
                            nc.scalar.copy(so[:, cs], po[:])
                    st_engs[blk % 3].dma_start(
                        out[r, nb * 128:(nb + 1) * 128, :], so[:])
                    blk += 1
    nc.compile()
    return nc


def _get_nc():
    global _NC_CACHE
    if _NC_CACHE is None:
        _NC_CACHE = _build()
    return _NC_CACHE


def kernel(**inputs):
    global LAST_RESULT
    A = np.asarray(inputs["adjacency"], dtype=np.float32)
    x0 = np.asarray(inputs["features"], dtype=np.float32)
    W = np.asarray(inputs["conv_weights"], dtype=np.float32)
    Mrel = np.asarray(inputs["rel_matrices"], dtype=np.float32)

    # h1[r, m, g] = sum_f x0[m, f] * W[0, r, g, f]; SBUF layout [p, r, mb, g].
    h1 = np.einsum("mf,rgf->rmg", x0, W[0])
    h1_tiled = np.ascontiguousarray(
        h1.reshape(R, MB, 128, F).transpose(2, 0, 1, 3)
    ).reshape(128, R * MB * F).astype(F8NP)
    # wt2[f, (r, g)] = W[1, r, g, f] / 4 (fp16 AR partial headroom)
    wt2 = np.ascontiguousarray(
        W[1].transpose(2, 0, 1) / Y2SCALE).reshape(F, R * F).astype(np.float16)
    # relm[g1, (r, g2)] = M[r, g1, g2]
    relm = np.ascontiguousarray(
        Mrel.transpose(1, 0, 2)).reshape(F, R * F).astype(np.float16)

    nc = _get_nc()
    in_maps = []
    for c in range(NCORES):
        sl = A[:, c * NL:(c + 1) * NL, :]             # [R, NL, N]
        atr = np.ascontiguousarray(
            sl.transpose(0, 2, 1)                      # [R, N(m), NL(j)]
            .reshape(R, MB, 128, NL)
            .transpose(0, 2, 1, 3)                     # [R, p, mb, j]
        ).astype(F8NP)
        slc = A[:, :, c * NL:(c + 1) * NL]            # [R, N(n), NL(m)]
        atc = np.ascontiguousarray(
            slc.transpose(0, 2, 1)                     # [R, NL(m), N(n)]
            .reshape(R, MBL, 128, N)
            .transpose(0, 2, 1, 3)                     # [R, p, mbl, n]
        ).astype(F8NP)
        idx = (np.arange(F, dtype=np.int32) * MC + c).reshape(F, 1)
        in_maps.append(dict(atr=atr, atc=atc, h1=h1_tiled, wt2=wt2,
                            relm=relm, idx=idx))

    res = bass_utils.run_bass_kernel_spmd(
        nc, in_maps, core_ids=list(range(NCORES)), trace=TRACE,
    )
    LAST_RESULT = res

    out = np.empty((R, N, N), dtype=np.float32)
    for c in range(NCORES):
        out[:, c * NL:(c + 1) * NL, :] = res.results[c]["out"]
    return out


# revision 15
# speedup vs baseline: 1.1015x; 1.1015x over previous
"""BasicRGCN Trainium2 kernel (8 NeuronCores, SPMD) - AllReduce formulation.

Math (reference):
    x = features                                   # [N, F]
    for l in 0..1:
        y = sum_r A[r] @ x @ W[l, r].T             # [N, F]
        x = sigmoid(y)
    out[r] = (x @ M_r) @ x.T                       # [R, N, N]

Layer 2 is computed as a PARTIAL SUM per core: core c holds both its
adjacency row-slab A[:, rows_c, :] (for layer 1 + its output slab) and an
adjacency column-slab A[:, :, cols_c]. After layer 1 each core projects only
its LOCAL x1 rows (h2loc = x1_loc @ W2.T, fp8) and computes
y2_partial[g, n] = sum_r sum_{m in cols_c} h2loc_r[m, g] A[r, n, m] for ALL
n. A fp16 ReduceScatter of y2_partial gives each core the summed g-rows of
its shard; the sigmoid runs once per shard and a fp16 AllGather of the
activated x2 reconstructs the full [F, N] x2 on every core - replacing the
two AllGathers (x1 and x2) of the row-parallel formulation, removing the
gathered-h2 recompute from the critical path, and paying much less than a
full AllReduce. Partials are pre-scaled by 1/4 (folded into W2 on the
host) so the fp16 CCE sum cannot overflow; the sigmoid applies scale=4.

Per-core identity (which columns are "mine" for the DistMult xm factor)
enters only through a tiny per-core int32 index input driving one indirect
DMA gather - the NEFF itself is identical on all 8 cores (SPMD).

Precision: fp8 adjacency/h1/h2loc with fp32 PSUM accumulation (layer-2
pre-activations are ~5e4 so sigmoid saturates and absorbs layer error);
fp16 x2/xm DistMult (worst-case rel err ~4e-5 in this regime).

Schedule notes (from ntff profiles of prior revisions):
  * The collectives runtime pays a ~40us init/launch-skew barrier (starts
    ~22us into the NEFF regardless of trigger time) plus ~11us first-op
    setup; both overlap the load/compute phase, so the ReduceScatter starts
    processing about when its data is ready.
  * Loads (17 MiB: row-slab, col-slab, h1) rotate across the three DGE
    rings (sync=SP-HWDGE, gpsimd=SWDGE, scalar=ACT-HWDGE); the phase is
    HBM-bound (~358 GB/s) at ~50us, overlapping L1 and the cc barrier.
  * Layer 1 and the layer-2 partials interleave 4 independent PSUM
    accumulation chains (a single chain serializes at ~390ns/matmul; 4
    chains pipeline at ~234ns), consuming slab blocks in DMA arrival
    order.
  * Output stores are one fully-contiguous 2 MiB DMA per 128-row block,
    rotated across the three DGE rings (~350 GB/s sustained, HBM-write
    bound - the phase floor).
  * No keep-warm is needed: the PE idles only across the AllReduce, and
    even at the cold-isolated matmul rate the DistMult (one K=64 fp16
    matmul per [128,512] tile) outpaces the stores.
"""

import numpy as np
import ml_dtypes

import concourse.bacc as bacc
import concourse.mybir as mybir
import concourse.tile as tile
from concourse import bass, bass_utils

R, N, F = 4, 4096, 64
NCORES = 8
NL = N // NCORES          # 512 local node rows per core
MB = N // 128             # 32 contraction blocks of 128 (layer 1)
MBL = NL // 128           # 4 local contraction blocks (layer 2 partial)
NB = NL // 128            # 4 output row-blocks per core
MC = N // 512             # 8 column-chunks

WARM0 = 16                # pre-warm matmuls at kernel start
Y2SCALE = 4.0             # fp16 AR partials carry y2/4; sigmoid re-scales

F8NP = ml_dtypes.float8_e4m3fn
F8 = mybir.dt.float8e4
F16 = mybir.dt.float16
F32 = mybir.dt.float32
I32 = mybir.dt.int32

# Set by the test harness to collect a profile; grading path leaves these alone.
TRACE = False
LAST_RESULT = None

_NC_CACHE = None


def _build():
    nc = bacc.Bacc("TRN2", target_bir_lowering=False, debug=False,
                   num_devices=NCORES)

    # Per-core inputs (host pre-laid-out; see kernel() below).
    atr = nc.dram_tensor("atr", [R, 128, MB, NL], F8, kind="ExternalInput")
    atc = nc.dram_tensor("atc", [R, 128, MBL, N], F8, kind="ExternalInput")
    h1 = nc.dram_tensor("h1", [128, R * MB * F], F8, kind="ExternalInput")
    wt2 = nc.dram_tensor("wt2", [F, R * F], F16, kind="ExternalInput")
    relm = nc.dram_tensor("relm", [F, R], F32, kind="ExternalInput")
    idx = nc.dram_tensor("idx", [F, 1], I32, kind="ExternalInput")
    out = nc.dram_tensor("out", [R, NL, N], F32, kind="ExternalOutput")

    rg = [list(range(NCORES))]
    SIG = mybir.ActivationFunctionType.Sigmoid

    with tile.TileContext(nc) as tc:
        with (
            tc.tile_pool(name="big", bufs=1) as big,
            tc.tile_pool(name="sb", bufs=1) as sb,
            tc.tile_pool(name="stage", bufs=3) as stage,
            tc.tile_pool(name="psl", bufs=4, space="PSUM") as psl,
            tc.tile_pool(name="psh", bufs=1, space="PSUM") as psh,
            tc.tile_pool(name="pso", bufs=3, space="PSUM") as pso,
            tc.tile_pool(name="xn", bufs=1) as xn,
            tc.tile_pool(name="dram", bufs=1, space="DRAM") as dram,
        ):
            # ---- CC warmup: trigger a dummy tiny AllGather first thing.
            # Nothing reads its output. The collectives runtime pays its
            # ~11us first-data-op setup at TRIGGER time (not barrier end),
            # so prepaying it here keeps the ReduceScatter off that cost.
            cw_in = dram.tile([F, 16], F16)
            cw_out = dram.tile([NCORES, F, 16], F16, addr_space="Shared")
            nc.gpsimd.collective_compute(
                "AllGather", mybir.AluOpType.bypass, replica_groups=rg,
                ins=[cw_in[:]], outs=[cw_out[:]],
            )

            # Resident adjacency slabs: rows (L1) + columns (L2 partial).
            a_res = big.tile([128, R * MB * NL], F8)
            a_v = a_res.rearrange("p (r mb j) -> p r mb j", r=R, mb=MB)
            a_col = big.tile([128, R * MBL * N], F8)
            ac_v = a_col.rearrange("p (r mb n) -> p r mb n", r=R, mb=MBL)

            h1_sb = sb.tile([128, R * MB * F], F8)
            h1_v = h1_sb.rearrange("p (r mb g) -> p r mb g", r=R, mb=MB)
            wt2_sb = sb.tile([F, R * F], F16)
            relm_sb = sb.tile([F, R], F32)
            idx_sb = sb.tile([F, 1], I32)

            rings = [nc.sync, nc.gpsimd, nc.scalar]
            qi = 0

            def ring():
                nonlocal qi
                e = rings[qi % 3]
                qi += 1
                return e

            ring().dma_start(wt2_sb[:], wt2[:])
            ring().dma_start(relm_sb[:], relm[:])
            ring().dma_start(idx_sb[:], idx[:])
            HC = R * MB * F // 4
            for q in range(4):
                ring().dma_start(h1_sb[:, q * HC:(q + 1) * HC],
                                 h1[:, q * HC:(q + 1) * HC])
            # Row-slab first (layer 1 consumes it), then column-slab.
            H = MB // 4
            for r in range(R):
                for h in range(4):
                    ring().dma_start(
                        a_v[:, r, h * H:(h + 1) * H, :],
                        atr[r, :, h * H:(h + 1) * H, :],
                    )
            for r in range(R):
                for mbl in range(MBL):
                    ring().dma_start(
                        ac_v[:, r, mbl, :],
                        atc[r, :, mbl, :],
                    )

            # Pre-warm the PE while the adjacency stream lands. Junk results
            # land in a psl ring buffer; layer 1 resets it with start=True.
            warm = psl.tile([F, NL], F32, tag="acc")
            for _ in range(WARM0):
                nc.tensor.matmul(warm[:], h1_v[:, 0, 0, :],
                                 h1_sb[:, 0:NL], start=True, stop=True)

            # ---- Layer 1: y1T[g, n_local] = sum_{r, m} h1_r[m, g] A[r, n, m]
            # 4 interleaved PSUM accumulation chains (avoids the serialized
            # single-chain matmul pace), combined on the vector engine.
            x1pack = sb.tile([F, NL], F16)
            y1c = [psl.tile([F, NL], F32, tag="acc", name=f"y1c{j}")
                   for j in range(4)]
            NK1 = R * MB // 2
            DR = mybir.MatmulPerfMode.DoubleRow
            k = 0
            for r in range(R):
                for mb in range(0, MB, 2):
                    nc.tensor.matmul(
                        y1c[k % 4][:], h1_v[:, r, mb:mb + 2, :],
                        a_v[:, r, mb:mb + 2, :],
                        start=(k < 4), stop=(k >= NK1 - 4), perf_mode=DR,
                    )
                    k += 1
            y1s = sb.tile([F, NL], F32)
            nc.vector.tensor_copy(y1s[:], y1c[0][:])
            nc.vector.tensor_add(y1s[:], y1s[:], y1c[1][:])
            nc.vector.tensor_add(y1s[:], y1s[:], y1c[2][:])
            nc.vector.tensor_add(y1s[:], y1s[:], y1c[3][:])
            nc.scalar.activation(x1pack[:], y1s[:], SIG)

            # ---- h2loc[m, (r g)] = x1_loc[m, :] @ W2r.T (local rows, fp8)
            h2loc = sb.tile([128, MBL * R * F], F8)
            h2l_v = h2loc.rearrange("p (mb r g) -> p mb r g", mb=MBL, r=R)
            for mbl in range(MBL):
                ph = psh.tile([128, R * F], F32, tag="h")
                nc.tensor.matmul(ph[:], x1pack[:, mbl * 128:(mbl + 1) * 128],
                                 wt2_sb[:], start=True, stop=True)
                ceng = nc.vector.tensor_copy if mbl % 2 == 0 else nc.scalar.copy
                ceng(h2l_v[:, mbl, :, :],
                     ph[:].rearrange("p (r g) -> p r g", r=R))

            # ---- Layer-2 partials: y2p[g, n] = sum_{r, m_loc} h2loc A_col.
            # Two n-halves; 4 interleaved PSUM accumulation chains per half,
            # consuming column-slab blocks in DMA arrival order.
            y2p = xn.tile([F, N], F16, tag="xn", name="y2p")
            for half in range(2):
                acc = [psl.tile([F, 512], F32, tag="acc", name=f"acc{half}_{j}")
                       for j in range(4)]
                k = 0
                for r in range(R):
                    for mbl in range(0, MBL, 2):
                        for j in range(4):
                            off = half * 2048 + j * 512
                            nc.tensor.matmul(
                                acc[j][:], h2l_v[:, mbl:mbl + 2, r, :],
                                ac_v[:, r, mbl:mbl + 2, off:off + 512],
                                start=(k == 0), stop=(k == R * MBL // 2 - 1),
                                perf_mode=DR,
                            )
                        k += 1
                for j in range(4):
                    off = half * 2048 + j * 512
                    ceng = (nc.vector.tensor_copy if j % 2 == 0
                            else nc.scalar.copy)
                    ceng(y2p[:, off:off + 512], acc[j][:])

            # ---- ReduceScatter y2 partials (fp16, scaled by 1/4): each core
            # receives the summed g-rows [8c:8c+8] for all n (the flat
            # scatter axis of the g-major [F, N] buffer).
            rs_in = dram.tile([F, N], F16)
            rs_out = dram.tile([F // NCORES, N], F16)
            nc.sync.dma_start(rs_in[:], y2p[:])
            nc.gpsimd.collective_compute(
                "ReduceScatter", mybir.AluOpType.add, replica_groups=rg,
                ins=[rs_in[:]], outs=[rs_out[:]],
            )

            # Sigmoid the own shard (scale undoes the 1/4 partial headroom),
            # then AllGather the activated fp16 x2 (rank-major g-rows concat
            # reconstructs x2[g, n] row-major exactly).
            ysh = sb.tile([F, 512], F16)
            nc.gpsimd.dma_start(
                ysh[:], rs_out[:].rearrange("a (b j) -> (a b) j", b=NCORES))
            xsh = sb.tile([F, 512], F16)
            nc.scalar.activation(xsh[:], ysh[:], SIG, scale=Y2SCALE)
            ag_in = dram.tile([F // NCORES, N], F16)
            ag_out = dram.tile([F, N], F16, addr_space="Shared")
            nc.scalar.dma_start(
                ag_in[:].rearrange("a (b j) -> (a b) j", b=NCORES), xsh[:])
            nc.gpsimd.collective_compute(
                "AllGather", mybir.AluOpType.bypass, replica_groups=rg,
                ins=[ag_in[:]], outs=[ag_out[:]],
            )

            # ---- Own-columns x2 slice via indirect gather (per-core idx).
            # ag_out viewed as [(g, chunk), 512]; row g*8+c -> partition g.
            x2own = sb.tile([F, NL], F16)
            ag_view = ag_out[:].rearrange("g (c j) -> (g c) j", c=MC)
            nc.gpsimd.indirect_dma_start(
                out=x2own[:], out_offset=None,
                in_=ag_view,
                in_offset=bass.IndirectOffsetOnAxis(ap=idx_sb[:, :1], axis=0),
            )

            # ---- Full x2: straight loads (already activated).
            x2t = xn.tile([F, N], F16, tag="xn", name="x2t")
            for q in range(MC):
                cs = slice(q * 512, (q + 1) * 512)
                rings[q % 3].dma_start(x2t[:, cs], ag_out[:, cs])

            # ---- xmT[r] = diag(M_r) * x2_own (rel_matrices are diagonal
            # by construction in this model; diag values shipped as [F, R]).
            xm16 = sb.tile([F, R * NL], F16)
            xm16_v = xm16.rearrange("g (r j) -> g r j", r=R)
            for r in range(R):
                if r % 2 == 0:
                    nc.vector.tensor_scalar_mul(
                        xm16_v[:, r, :], x2own[:], relm_sb[:, r:r + 1])
                else:
                    nc.scalar.mul(xm16_v[:, r, :], x2own[:],
                                  relm_sb[:, r:r + 1])

            # ---- DistMult scores: out[r, n, m] = sum_g xm[r][n, g] x2[m, g]
            # One K=64 fp16 matmul per [128, 512] tile; stage a full 128-row
            # block in SBUF, store it as ONE contiguous 2 MiB DMA, rotating
            # across the three DGE rings.
            st_engs = [nc.sync, nc.gpsimd, nc.scalar]
            blk = 0
            for r in range(R):
                for nb in range(NB):
                    lhs = xm16_v[:, r, nb * 128:(nb + 1) * 128]
                    so = stage.tile([128, N], F32, tag="so")
                    for mc in range(MC):
                        cs = slice(mc * 512, (mc + 1) * 512)
                        po = pso.tile([128, 512], F32, tag="o")
                        nc.tensor.matmul(po[:], lhs, x2t[:, cs],
                                         start=True, stop=True)
                        if mc % 2 == 0:
                            nc.vector.tensor_copy(so[:, cs], po[:])
                        else:
                            nc.scalar.copy(so[:, cs], po[:])
                    st_engs[blk % 3].dma_start(
                        out[r, nb * 128:(nb + 1) * 128, :], so[:])
                    blk += 1
    nc.compile()
    return nc


def _get_nc():
    global _NC_CACHE
    if _NC_CACHE is None:
        _NC_CACHE = _build()
    return _NC_CACHE


def kernel(**inputs):
    global LAST_RESULT
    A = np.asarray(inputs["adjacency"], dtype=np.float32)
    x0 = np.asarray(inputs["features"], dtype=np.float32)
    W = np.asarray(inputs["conv_weights"], dtype=np.float32)
    Mrel = np.asarray(inputs["rel_matrices"], dtype=np.float32)

    # h1[r, m, g] = sum_f x0[m, f] * W[0, r, g, f]; SBUF layout [p, r, mb, g].
    h1 = np.einsum("mf,rgf->rmg", x0, W[0])
    h1_tiled = np.ascontiguousarray(
        h1.reshape(R, MB, 128, F).transpose(2, 0, 1, 3)
    ).reshape(128, R * MB * F).astype(F8NP)
    # wt2[f, (r, g)] = W[1, r, g, f] / 4 (fp16 AR partial headroom)
    wt2 = np.ascontiguousarray(
        W[1].transpose(2, 0, 1) / Y2SCALE).reshape(F, R * F).astype(np.float16)
    # relm[g, r] = M[r, g, g] (rel_matrices are diagonal by construction)
    relm = np.ascontiguousarray(
        np.diagonal(Mrel, axis1=1, axis2=2).T).astype(np.float32)

    nc = _get_nc()
    in_maps = []
    for c in range(NCORES):
        sl = A[:, c * NL:(c + 1) * NL, :]             # [R, NL, N]
        atr = np.ascontiguousarray(
            sl.transpose(0, 2, 1)                      # [R, N(m), NL(j)]
            .reshape(R, MB, 128, NL)
            .transpose(0, 2, 1, 3)                     # [R, p, mb, j]
        ).astype(F8NP)
        slc = A[:, :, c * NL:(c + 1) * NL]            # [R, N(n), NL(m)]
        atc = np.ascontiguousarray(
            slc.transpose(0, 2, 1)                     # [R, NL(m), N(n)]
            .reshape(R, MBL, 128, N)
            .transpose(0, 2, 1, 3)                     # [R, p, mbl, n]
        ).astype(F8NP)
        idx = (np.arange(F, dtype=np.int32) * MC + c).reshape(F, 1)
        in_maps.append(dict(atr=atr, atc=atc, h1=h1_tiled, wt2=wt2,
                            relm=relm, idx=idx))

    res = bass_utils.run_bass_kernel_spmd(
        nc, in_maps, core_ids=list(range(NCORES)), trace=TRACE,
    )
    LAST_RESULT = res

    out = np.empty((R, N, N), dtype=np.float32)
    for c in range(NCORES):
        out[:, c * NL:(c + 1) * NL, :] = res.results[c]["out"]
    return out


# revision 16
# speedup vs baseline: 1.1073x; 1.0053x over previous
"""BasicRGCN Trainium2 kernel (8 NeuronCores, SPMD) - AllReduce formulation.

Math (reference):
    x = features                                   # [N, F]
    for l in 0..1:
        y = sum_r A[r] @ x @ W[l, r].T             # [N, F]
        x = sigmoid(y)
    out[r] = (x @ M_r) @ x.T                       # [R, N, N]

Layer 2 is computed as a PARTIAL SUM per core: core c holds both its
adjacency row-slab A[:, rows_c, :] (for layer 1 + its output slab) and an
adjacency column-slab A[:, :, cols_c]. After layer 1 each core projects only
its LOCAL x1 rows (h2loc = x1_loc @ W2.T, fp8) and computes
y2_partial[g, n] = sum_r sum_{m in cols_c} h2loc_r[m, g] A[r, n, m] for ALL
n. A fp16 ReduceScatter of y2_partial gives each core the summed g-rows of
its shard; the sigmoid runs once per shard and a fp16 AllGather of the
activated x2 reconstructs the full [F, N] x2 on every core - replacing the
two AllGathers (x1 and x2) of the row-parallel formulation, removing the
gathered-h2 recompute from the critical path, and paying much less than a
full AllReduce. Partials are pre-scaled by 1/4 (folded into W2 on the
host) so the fp16 CCE sum cannot overflow; the sigmoid applies scale=4.

Per-core identity (which columns are "mine" for the DistMult xm factor)
enters only through a tiny per-core int32 index input driving one indirect
DMA gather - the NEFF itself is identical on all 8 cores (SPMD).

Precision: fp8 adjacency/h1/h2loc with fp32 PSUM accumulation (layer-2
pre-activations are ~5e4 so sigmoid saturates and absorbs layer error);
fp16 x2/xm DistMult (worst-case rel err ~4e-5 in this regime).

Schedule notes (from ntff profiles of prior revisions):
  * The collectives runtime pays a ~40us init/launch-skew barrier (starts
    ~22us into the NEFF regardless of trigger time) plus ~11us first-op
    setup; both overlap the load/compute phase, so the ReduceScatter starts
    processing about when its data is ready.
  * Loads (17 MiB: row-slab, col-slab, h1) rotate across the three DGE
    rings (sync=SP-HWDGE, gpsimd=SWDGE, scalar=ACT-HWDGE); the phase is
    HBM-bound (~358 GB/s) at ~50us, overlapping L1 and the cc barrier.
  * Layer 1 and the layer-2 partials interleave 4 independent PSUM
    accumulation chains (a single chain serializes at ~390ns/matmul; 4
    chains pipeline at ~234ns), consuming slab blocks in DMA arrival
    order.
  * Output stores are one fully-contiguous 2 MiB DMA per 128-row block,
    rotated across the three DGE rings (~350 GB/s sustained, HBM-write
    bound - the phase floor).
  * No keep-warm is needed: the PE idles only across the AllReduce, and
    even at the cold-isolated matmul rate the DistMult (one K=64 fp16
    matmul per [128,512] tile) outpaces the stores.
"""

import numpy as np
import ml_dtypes

import concourse.bacc as bacc
import concourse.mybir as mybir
import concourse.tile as tile
from concourse import bass, bass_utils

R, N, F = 4, 4096, 64
NCORES = 8
NL = N // NCORES          # 512 local node rows per core
MB = N // 128             # 32 contraction blocks of 128 (layer 1)
MBL = NL // 128           # 4 local contraction blocks (layer 2 partial)
NB = NL // 128            # 4 output row-blocks per core
MC = N // 512             # 8 column-chunks

WARM0 = 16                # pre-warm matmuls at kernel start
Y2SCALE = 4.0             # fp16 AR partials carry y2/4; sigmoid re-scales

F8NP = ml_dtypes.float8_e4m3fn
F8 = mybir.dt.float8e4
F16 = mybir.dt.float16
F32 = mybir.dt.float32
I32 = mybir.dt.int32

# Set by the test harness to collect a profile; grading path leaves these alone.
TRACE = False
LAST_RESULT = None

_NC_CACHE = None


def _build():
    nc = bacc.Bacc("TRN2", target_bir_lowering=False, debug=False,
                   num_devices=NCORES)

    # Per-core inputs (host pre-laid-out; see kernel() below).
    atr = nc.dram_tensor("atr", [R, 128, MB, NL], F8, kind="ExternalInput")
    atc = nc.dram_tensor("atc", [R, 128, MBL, N], F8, kind="ExternalInput")
    h1 = nc.dram_tensor("h1", [128, R * MB * F], F8, kind="ExternalInput")
    wt2 = nc.dram_tensor("wt2", [F, R * F], F16, kind="ExternalInput")
    relm = nc.dram_tensor("relm", [F, R], F32, kind="ExternalInput")
    idx = nc.dram_tensor("idx", [F, 1], I32, kind="ExternalInput")
    out = nc.dram_tensor("out", [R, NL, N], F32, kind="ExternalOutput")

    rg = [list(range(NCORES))]
    SIG = mybir.ActivationFunctionType.Sigmoid

    with tile.TileContext(nc) as tc:
        with (
            tc.tile_pool(name="big", bufs=1) as big,
            tc.tile_pool(name="sb", bufs=1) as sb,
            tc.tile_pool(name="stage", bufs=3) as stage,
            tc.tile_pool(name="psl", bufs=4, space="PSUM") as psl,
            tc.tile_pool(name="psh", bufs=1, space="PSUM") as psh,
            tc.tile_pool(name="pso", bufs=3, space="PSUM") as pso,
            tc.tile_pool(name="xn", bufs=1) as xn,
            tc.tile_pool(name="dram", bufs=1, space="DRAM") as dram,
        ):
            # ---- CC warmup: trigger a dummy tiny AllGather first thing.
            # Nothing reads its output. The collectives runtime pays its
            # ~11us first-data-op setup at TRIGGER time (not barrier end),
            # so prepaying it here keeps the ReduceScatter off that cost.
            cw_in = dram.tile([F, 16], F16)
            cw_out = dram.tile([NCORES, F, 16], F16, addr_space="Shared")
            nc.gpsimd.collective_compute(
                "AllGather", mybir.AluOpType.bypass, replica_groups=rg,
                ins=[cw_in[:]], outs=[cw_out[:]],
            )

            # Resident adjacency slabs: rows (L1) + columns (L2 partial).
            a_res = big.tile([128, R * MB * NL], F8)
            a_v = a_res.rearrange("p (r mb j) -> p r mb j", r=R, mb=MB)
            a_col = big.tile([128, R * MBL * N], F8)
            ac_v = a_col.rearrange("p (r mb n) -> p r mb n", r=R, mb=MBL)

            h1_sb = sb.tile([128, R * MB * F], F8)
            h1_v = h1_sb.rearrange("p (r mb g) -> p r mb g", r=R, mb=MB)
            wt2_sb = sb.tile([F, R * F], F16)
            relm_sb = sb.tile([F, R], F32)
            idx_sb = sb.tile([F, 1], I32)

            rings = [nc.sync, nc.gpsimd, nc.scalar]
            qi = 0

            def ring():
                nonlocal qi
                e = rings[qi % 3]
                qi += 1
                return e

            ring().dma_start(wt2_sb[:], wt2[:])
            ring().dma_start(relm_sb[:], relm[:])
            ring().dma_start(idx_sb[:], idx[:])
            HC = R * MB * F // 2
            for q in range(2):
                ring().dma_start(h1_sb[:, q * HC:(q + 1) * HC],
                                 h1[:, q * HC:(q + 1) * HC])
            # Row-slab first (layer 1 consumes it), then column-slab.
            # 1 MiB per DMA: per-ring FIFO pays ~2us completion latency per
            # transfer, so fewer/bigger transfers raise the aggregate rate.
            H = MB // 2
            for r in range(R):
                for h in range(2):
                    ring().dma_start(
                        a_v[:, r, h * H:(h + 1) * H, :],
                        atr[r, :, h * H:(h + 1) * H, :],
                    )
            for r in range(R):
                for mbl in range(0, MBL, 2):
                    ring().dma_start(
                        ac_v[:, r, mbl:mbl + 2, :],
                        atc[r, :, mbl:mbl + 2, :],
                    )

            # Pre-warm the PE while the adjacency stream lands. Junk results
            # land in a psl ring buffer; layer 1 resets it with start=True.
            warm = psl.tile([F, NL], F32, tag="acc")
            for _ in range(WARM0):
                nc.tensor.matmul(warm[:], h1_v[:, 0, 0, :],
                                 h1_sb[:, 0:NL], start=True, stop=True)

            # ---- Layer 1: y1T[g, n_local] = sum_{r, m} h1_r[m, g] A[r, n, m]
            # 4 interleaved PSUM accumulation chains (avoids the serialized
            # single-chain matmul pace), combined on the vector engine.
            x1pack = sb.tile([F, NL], F16)
            y1c = [psl.tile([F, NL], F32, tag="acc", name=f"y1c{j}")
                   for j in range(4)]
            NK1 = R * MB // 2
            DR = mybir.MatmulPerfMode.DoubleRow
            k = 0
            for r in range(R):
                for mb in range(0, MB, 2):
                    nc.tensor.matmul(
                        y1c[k % 4][:], h1_v[:, r, mb:mb + 2, :],
                        a_v[:, r, mb:mb + 2, :],
                        start=(k < 4), stop=(k >= NK1 - 4), perf_mode=DR,
                    )
                    k += 1
            y1s = sb.tile([F, NL], F32)
            nc.vector.tensor_copy(y1s[:], y1c[0][:])
            nc.vector.tensor_add(y1s[:], y1s[:], y1c[1][:])
            nc.vector.tensor_add(y1s[:], y1s[:], y1c[2][:])
            nc.vector.tensor_add(y1s[:], y1s[:], y1c[3][:])
            nc.scalar.activation(x1pack[:], y1s[:], SIG)

            # ---- h2loc[m, (r g)] = x1_loc[m, :] @ W2r.T (local rows, fp8)
            h2loc = sb.tile([128, MBL * R * F], F8)
            h2l_v = h2loc.rearrange("p (mb r g) -> p mb r g", mb=MBL, r=R)
            for mbl in range(MBL):
                ph = psh.tile([128, R * F], F32, tag="h")
                nc.tensor.matmul(ph[:], x1pack[:, mbl * 128:(mbl + 1) * 128],
                                 wt2_sb[:], start=True, stop=True)
                ceng = nc.vector.tensor_copy if mbl % 2 == 0 else nc.scalar.copy
                ceng(h2l_v[:, mbl, :, :],
                     ph[:].rearrange("p (r g) -> p r g", r=R))

            # ---- Layer-2 partials: y2p[g, n] = sum_{r, m_loc} h2loc A_col.
            # Two n-halves; 4 interleaved PSUM accumulation chains per half,
            # consuming column-slab blocks in DMA arrival order.
            y2p = xn.tile([F, N], F16, tag="xn", name="y2p")
            for half in range(2):
                acc = [psl.tile([F, 512], F32, tag="acc", name=f"acc{half}_{j}")
                       for j in range(4)]
                k = 0
                for r in range(R):
                    for mbl in range(0, MBL, 2):
                        for j in range(4):
                            off = half * 2048 + j * 512
                            nc.tensor.matmul(
                                acc[j][:], h2l_v[:, mbl:mbl + 2, r, :],
                                ac_v[:, r, mbl:mbl + 2, off:off + 512],
                                start=(k == 0), stop=(k == R * MBL // 2 - 1),
                                perf_mode=DR,
                            )
                        k += 1
                for j in range(4):
                    off = half * 2048 + j * 512
                    ceng = (nc.vector.tensor_copy if j % 2 == 0
                            else nc.scalar.copy)
                    ceng(y2p[:, off:off + 512], acc[j][:])

            # ---- ReduceScatter y2 partials (fp16, scaled by 1/4): each core
            # receives the summed g-rows [8c:8c+8] for all n (the flat
            # scatter axis of the g-major [F, N] buffer).
            rs_in = dram.tile([F, N], F16)
            rs_out = dram.tile([F // NCORES, N], F16)
            nc.sync.dma_start(rs_in[:, 0:2048], y2p[:, 0:2048])
            nc.sync.dma_start(rs_in[:, 2048:N], y2p[:, 2048:N])
            nc.gpsimd.collective_compute(
                "ReduceScatter", mybir.AluOpType.add, replica_groups=rg,
                ins=[rs_in[:]], outs=[rs_out[:]],
            )

            # Sigmoid the own shard (scale undoes the 1/4 partial headroom),
            # then AllGather the activated fp16 x2 (rank-major g-rows concat
            # reconstructs x2[g, n] row-major exactly).
            ysh = sb.tile([F, 512], F16)
            nc.gpsimd.dma_start(
                ysh[:], rs_out[:].rearrange("a (b j) -> (a b) j", b=NCORES))
            xsh = sb.tile([F, 512], F16)
            nc.scalar.activation(xsh[:], ysh[:], SIG, scale=Y2SCALE)
            ag_in = dram.tile([F // NCORES, N], F16)
            ag_out = dram.tile([F, N], F16, addr_space="Shared")
            nc.scalar.dma_start(
                ag_in[:].rearrange("a (b j) -> (a b) j", b=NCORES), xsh[:])
            nc.gpsimd.collective_compute(
                "AllGather", mybir.AluOpType.bypass, replica_groups=rg,
                ins=[ag_in[:]], outs=[ag_out[:]],
            )

            # ---- Own-columns x2 slice via indirect gather (per-core idx).
            # ag_out viewed as [(g, chunk), 512]; row g*8+c -> partition g.
            x2own = sb.tile([F, NL], F16)
            ag_view = ag_out[:].rearrange("g (c j) -> (g c) j", c=MC)
            nc.gpsimd.indirect_dma_start(
                out=x2own[:], out_offset=None,
                in_=ag_view,
                in_offset=bass.IndirectOffsetOnAxis(ap=idx_sb[:, :1], axis=0),
            )

            # ---- Full x2: straight loads (already activated).
            x2t = xn.tile([F, N], F16, tag="xn", name="x2t")
            for q in range(MC):
                cs = slice(q * 512, (q + 1) * 512)
                rings[q % 3].dma_start(x2t[:, cs], ag_out[:, cs])

            # ---- xmT[r] = diag(M_r) * x2_own (rel_matrices are diagonal
            # by construction in this model; diag values shipped as [F, R]).
            xm16 = sb.tile([F, R * NL], F16)
            xm16_v = xm16.rearrange("g (r j) -> g r j", r=R)
            for r in range(R):
                if r % 2 == 0:
                    nc.vector.tensor_scalar_mul(
                        xm16_v[:, r, :], x2own[:], relm_sb[:, r:r + 1])
                else:
                    nc.scalar.mul(xm16_v[:, r, :], x2own[:],
                                  relm_sb[:, r:r + 1])

            # ---- DistMult scores: out[r, n, m] = sum_g xm[r][n, g] x2[m, g]
            # One K=64 fp16 matmul per [128, 512] tile; stage a full 128-row
            # block in SBUF, store it as ONE contiguous 2 MiB DMA, rotating
            # across the three DGE rings.
            st_engs = [nc.sync, nc.gpsimd, nc.scalar]
            blk = 0
            for r in range(R):
                for nb in range(NB):
                    lhs = xm16_v[:, r, nb * 128:(nb + 1) * 128]
                    so = stage.tile([128, N], F32, tag="so")
                    for mc in range(MC):
                        cs = slice(mc * 512, (mc + 1) * 512)
                        po = pso.tile([128, 512], F32, tag="o")
                        nc.tensor.matmul(po[:], lhs, x2t[:, cs],
                                         start=True, stop=True)
                        if mc % 2 == 0:
                            nc.vector.tensor_copy(so[:, cs], po[:])
                        else:
                            nc.scalar.copy(so[:, cs], po[:])
                    st_engs[blk % 3].dma_start(
                        out[r, nb * 128:(nb + 1) * 128, :], so[:])
                    blk += 1
    nc.compile()
    return nc


def _get_nc():
    global _NC_CACHE
    if _NC_CACHE is None:
        _NC_CACHE = _build()
    return _NC_CACHE


def kernel(**inputs):
    global LAST_RESULT
    A = np.asarray(inputs["adjacency"], dtype=np.float32)
    x0 = np.asarray(inputs["features"], dtype=np.float32)
    W = np.asarray(inputs["conv_weights"], dtype=np.float32)
    Mrel = np.asarray(inputs["rel_matrices"], dtype=np.float32)

    # h1[r, m, g] = sum_f x0[m, f] * W[0, r, g, f]; SBUF layout [p, r, mb, g].
    h1 = np.einsum("mf,rgf->rmg", x0, W[0])
    h1_tiled = np.ascontiguousarray(
        h1.reshape(R, MB, 128, F).transpose(2, 0, 1, 3)
    ).reshape(128, R * MB * F).astype(F8NP)
    # wt2[f, (r, g)] = W[1, r, g, f] / 4 (fp16 AR partial headroom)
    wt2 = np.ascontiguousarray(
        W[1].transpose(2, 0, 1) / Y2SCALE).reshape(F, R * F).astype(np.float16)
    # relm[g, r] = M[r, g, g] (rel_matrices are diagonal by construction)
    relm = np.ascontiguousarray(
        np.diagonal(Mrel, axis1=1, axis2=2).T).astype(np.float32)

    nc = _get_nc()
    in_maps = []
    for c in range(NCORES):
        sl = A[:, c * NL:(c + 1) * NL, :]             # [R, NL, N]
        atr = np.ascontiguousarray(
            sl.transpose(0, 2, 1)                      # [R, N(m), NL(j)]
            .reshape(R, MB, 128, NL)
            .transpose(0, 2, 1, 3)                     # [R, p, mb, j]
        ).astype(F8NP)
        slc = A[:, :, c * NL:(c + 1) * NL]            # [R, N(n), NL(m)]
        atc = np.ascontiguousarray(
            slc.transpose(0, 2, 1)                     # [R, NL(m), N(n)]
            .reshape(R, MBL, 128, N)
            .transpose(0, 2, 1, 3)                     # [R, p, mbl, n]
        ).astype(F8NP)
        idx = (np.arange(F, dtype=np.int32) * MC + c).reshape(F, 1)
        in_maps.append(dict(atr=atr, atc=atc, h1=h1_tiled, wt2=wt2,
                            relm=relm, idx=idx))

    res = bass_utils.run_bass_kernel_spmd(
        nc, in_maps, core_ids=list(range(NCORES)), trace=TRACE,
    )
    LAST_RESULT = res

    out = np.empty((R, N, N), dtype=np.float32)
    for c in range(NCORES):
        out[:, c * NL:(c + 1) * NL, :] = res.results[c]["out"]
    return out
